# revision 15
# baseline (speedup 1.0000x reference)
"""Local cross-attention Trainium2 kernel.

Strategy (8 NeuronCores, SPMD):
  - Queries split into 32 kd-tree leaves of 128 ("slots"); each leaf
    gathers the EXACT union of its queries' neighborhoods (reference-mask
    semantics), padded to 128-multiples with sentinel keys.
  - Leaves sorted by padded chunk count; rank-group i (ranks 8i..8i+7)
    becomes slot i, one leaf per core, so per-slot loop bounds are uniform
    and minimal (sum_i max_c chunks is optimal for consecutive grouping).
  - Device per (slot, key-chunk): dist^2 via augmented-coords matmul (fp32,
    batched 4 chunks per PSUM bank); mask = (d2 <= 9) on DVE; scores via
    row-tiled K=32 matmuls (4 concurrent PE row-tiles -> 4 PSUM banks);
    E = exp(s/sqrt(32)) on ACT over all 8 heads at once; mask-mult on DVE;
    AV with ones-augmented V (M=33, 2-way col-tiled) accumulating output
    and softmax denominator in PSUM; per-slot normalize (reciprocal + PE
    broadcast + Pool multiplies); final output projection.
  - Host gathers outputs back to original query order.
"""
import sys, os
sys.path.insert(0, '/opt/trn_rl_repo')

import numpy as np
from contextlib import ExitStack

import ml_dtypes

F = 256           # feature dim
H = 8             # heads
D = 32            # head dim
R = 3.0
R2 = 9.0
NC = 8            # cores
P = 128
QS = 128          # queries per slot
NSLOT = 4         # slots per core (512 q / core)
NQ = NSLOT * QS
SENT = 1.0e4      # sentinel coordinate for padded keys

bf16 = ml_dtypes.bfloat16


# ---------------------------------------------------------------- host staging
def _leaves(cc, mask):
    """Split all queries into 32 kd leaves of 128; at each node pick the
    median split (of the 3 dims) minimizing the children's padded key-chunk
    total (exact neighborhood unions)."""
    leaves = [np.arange(cc.shape[0])]
    while len(leaves) < NC * NSLOT:
        nxt = []
        for l in leaves:
            pts = cc[l]
            best = None
            for d in range(3):
                order = np.argsort(pts[:, d], kind='stable')
                half = len(l) // 2
                l0, l1 = l[order[:half]], l[order[half:]]
                w0 = int(mask[l0].any(0).sum())
                w1 = int(mask[l1].any(0).sum())
                key = -(-w0 // P) + -(-w1 // P)
                if best is None or key < best[0]:
                    best = (key, l0, l1)
            nxt.append(best[1])
            nxt.append(best[2])
        leaves = nxt
    return leaves


def _plan(cc, hc):
    """kd leaves + exact-union key windows + rank-grouped slot assignment."""
    # reference-semantics mask (same float32 expression as reference())
    mask = np.zeros((cc.shape[0], hc.shape[0]), bool)
    for q0 in range(0, cc.shape[0], 512):
        d2 = ((cc[q0:q0+512, None, :] - hc[None, :, :]) ** 2).sum(
            -1, dtype=np.float32)
        mask[q0:q0+512] = d2 <= R2
    leaves = _leaves(cc, mask)
    sels = [np.nonzero(mask[l].any(0))[0] for l in leaves]
    chunks = np.array([max(1, (len(s) + P - 1) // P) for s in sels])
    order = np.argsort(-chunks, kind='stable')
    cores = [[] for _ in range(NC)]
    KW = []
    for i in range(NSLOT):
        grp = order[i * NC:(i + 1) * NC]
        KW.append(int(chunks[grp[0]]))
        for c in range(NC):
            li = grp[c]
            cores[c].append((leaves[li], sels[li]))
    return cores, KW


def _borderline(cc, hc):
    """Pairs whose exact (reference-form) dist^2 is within EPS of R2; the
    device dot-form matmul could round these to the wrong side of the mask
    boundary. Returns {k: [(q, delta), ...]} with delta pushing d2 safely
    to the reference side."""
    EPS = 1.5e-4
    out = {}
    for q0 in range(0, cc.shape[0], 512):
        d2 = ((cc[q0:q0+512, None, :] - hc[None, :, :]) ** 2).sum(-1,
                                                                  dtype=np.float32)
        qq, kk = np.nonzero(np.abs(d2 - R2) < EPS)
        for q, k in zip(qq, kk):
            delta = -5e-3 if d2[q, k] <= R2 else 5e-3
            out.setdefault(k, []).append((q0 + int(q), delta))
    return out


def _stage(inputs):
    cc = np.ascontiguousarray(np.asarray(inputs['current_coords'], np.float32))
    hc = np.ascontiguousarray(np.asarray(inputs['historical_coords'], np.float32))
    cf = np.asarray(inputs['current_feats'], np.float32)
    hf = np.asarray(inputs['historical_feats'], np.float32)

    cores, KW = _plan(cc, hc)
    NKP = sum(KW) * P          # padded key-instances per core
    border = _borderline(cc, hc)
    # max correction rows over cores, uniform NAUG
    ncorr = []
    for c in range(NC):
        subs = cores[c]
        rows = set()
        for i, (qs, sel) in enumerate(subs):
            qset = {int(q) for q in qs}
            for j, k in enumerate(sel):
                if int(k) in border and any(q in qset for q, _ in border[int(k)]):
                    rows.add((i, j))
        ncorr.append(len(rows))
    NAUG = 5 + max(max(ncorr), 1)

    # weights (shared across cores)
    WqT = np.ascontiguousarray(np.asarray(inputs['Wq'], np.float32).T).astype(bf16)
    WkT = np.ascontiguousarray(np.asarray(inputs['Wk'], np.float32).T).astype(bf16)
    WvT = np.ascontiguousarray(np.asarray(inputs['Wv'], np.float32).T).astype(bf16)
    WoT = np.ascontiguousarray(np.asarray(inputs['Wo'], np.float32).T).astype(bf16)
    bq = np.asarray(inputs['bq'], np.float32)
    bk = np.asarray(inputs['bk'], np.float32)
    bv = np.asarray(inputs['bv'], np.float32)
    bo = np.asarray(inputs['bo'], np.float32)
    bqk = np.stack([bq[:P], bq[P:], bk[:P], bk[P:]], 1)        # [128, 4]
    boT = np.stack([bo[:P], bo[P:]], 1)                        # [128, 2]
    bv_row = bv[None, :].astype(bf16)                          # [1, 256]
    vbias = bool(np.any(bv != 0.0))
    # Wo rows permuted to the epilogue's (parity, block) AV layout:
    # WoP[64*(h%2)+d, h//2, e] = Wo[e, 32*h+d]; dead rows zero.
    WoP = np.zeros((P, 4, F), np.float32)
    for h in range(H):
        rho, b = h % 2, h // 2
        WoP[64*rho:64*rho+D, b, :] = WoT[32*h:32*h+D, :].astype(np.float32)
    WoP = np.ascontiguousarray(WoP.reshape(P, 4*F)).astype(bf16)

    in_maps = []
    qmaps = []          # original query indices in slot order, per core
    for c in range(NC):
        subs = cores[c]
        qsel = np.concatenate([s[0] for s in subs])
        qmaps.append(qsel)
        # key-instance arrays
        kfeat = np.zeros((NKP, F), np.float32)
        kcoord = np.full((NKP, 3), SENT, np.float32)
        off = 0
        for i, (qs, sel) in enumerate(subs):
            kfeat[off:off + len(sel)] = hf[sel]
            kcoord[off:off + len(sel)] = hc[sel]
            off += KW[i] * P
        qc = cc[qsel]
        haug = np.zeros((NAUG, NKP), np.float32)
        haug[0:3] = kcoord.T
        haug[3] = (kcoord ** 2).sum(1)
        haug[4] = 1.0
        qaug = np.zeros((NAUG, len(qsel)), np.float32)
        qaug[0:3] = -2 * qc.T
        qaug[3] = 1.0
        qaug[4] = (qc ** 2).sum(1)
        # borderline corrections: one aug row per affected key instance
        row = 5
        off = 0
        for i, (qs, sel) in enumerate(subs):
            qlocal = {int(q): i * QS + j for j, q in enumerate(qs)}
            for j, k in enumerate(sel):
                if int(k) in border:
                    fixes = [(qlocal[q], d) for q, d in border[int(k)]
                             if q in qlocal]
                    if fixes:
                        haug[row, off + j] = 1.0
                        for qloc, d in fixes:
                            qaug[row, qloc] = d
                        row += 1
            off += KW[i] * P
        in_maps.append({
            'histTf': np.ascontiguousarray(kfeat.T).astype(bf16),
            'haug': np.ascontiguousarray(haug.astype(np.float32)),
            'curT': np.ascontiguousarray(cf[qsel].T).astype(bf16),
            'qaug': np.ascontiguousarray(qaug.astype(np.float32)),
            'wqT': WqT, 'wkT': WkT, 'wvT': WvT, 'woP': WoP,
            'bqk': bqk, 'boT': boT, 'bv_row': bv_row,
        })
    return in_maps, qmaps, KW, NKP, NAUG, vbias


# ---------------------------------------------------------------- bass kernel
def _build(KW, NKP, NAUG, vbias=False, reps=1):
    import concourse.bass as bass
    import concourse.bacc as bacc
    import concourse.tile as tile
    from concourse import mybir

    f32 = mybir.dt.float32
    b16 = mybir.dt.bfloat16
    NCH = NKP // P
    ISCALE = 1.0 / np.sqrt(D)

    nc = bacc.Bacc("TRN2", target_bir_lowering=False, debug=False,
                   enable_asserts=False, num_devices=NC)

    t_histTf = nc.dram_tensor('histTf', [F, NKP], b16, kind='ExternalInput')
    t_haug = nc.dram_tensor('haug', [NAUG, NKP], f32, kind='ExternalInput')
    t_curT = nc.dram_tensor('curT', [F, NQ], b16, kind='ExternalInput')
    t_qaug = nc.dram_tensor('qaug', [NAUG, NQ], f32, kind='ExternalInput')
    t_wqT = nc.dram_tensor('wqT', [F, F], b16, kind='ExternalInput')
    t_wkT = nc.dram_tensor('wkT', [F, F], b16, kind='ExternalInput')
    t_wvT = nc.dram_tensor('wvT', [F, F], b16, kind='ExternalInput')
    t_woP = nc.dram_tensor('woP', [P, 4 * F], b16, kind='ExternalInput')
    t_bqk = nc.dram_tensor('bqk', [P, 4], f32, kind='ExternalInput')
    t_boT = nc.dram_tensor('boT', [P, 2], f32, kind='ExternalInput')
    t_bv = nc.dram_tensor('bv_row', [1, F], b16, kind='ExternalInput')
    t_yT = nc.dram_tensor('yT', [F, NQ], f32, kind='ExternalOutput')

    base = np.cumsum([0] + KW)          # chunk base per slot

    with tile.TileContext(nc) as tc, ExitStack() as ctx:
        sing = ctx.enter_context(tc.tile_pool(name='sing', bufs=1))
        epool = ctx.enter_context(tc.tile_pool(name='epool', bufs=3))
        mpool = ctx.enter_context(tc.tile_pool(name='mpool', bufs=2))
        opool = ctx.enter_context(tc.tile_pool(name='opool', bufs=2))
        ps_sc = ctx.enter_context(tc.tile_pool(name='ps_sc', bufs=2, space='PSUM'))
        ps_d2 = ctx.enter_context(tc.tile_pool(name='ps_d2', bufs=1, space='PSUM'))
        ps_av = ctx.enter_context(tc.tile_pool(name='ps_av', bufs=2, space='PSUM'))
        ps_rb = ctx.enter_context(tc.tile_pool(name='ps_rb', bufs=1, space='PSUM'))

        for _rep in range(reps):
            _emit_once(nc, tc, mybir, KW, NKP, NAUG, base, NCH, ISCALE, vbias,
                       sing, epool, mpool, opool, ps_sc, ps_d2, ps_av, ps_rb,
                       t_histTf, t_haug, t_curT, t_qaug, t_wqT, t_wkT, t_wvT,
                       t_woP, t_bqk, t_boT, t_bv, t_yT, f32, b16)

    nc.compile()
    return nc


def _emit_once(nc, tc, mybir, KW, NKP, NAUG, base, NCH, ISCALE, vbias,
               sing, epool, mpool, opool, ps_sc, ps_d2, ps_av, ps_rb,
               t_histTf, t_haug, t_curT, t_qaug, t_wqT, t_wkT, t_wvT,
               t_woP, t_bqk, t_boT, t_bv, t_yT, f32, b16):
    Exp = mybir.ActivationFunctionType.Exp
    Ident = mybir.ActivationFunctionType.Identity

    # ---------------- load inputs (spread across 4 DMA queues)
    sb_hist = [sing.tile([P, NKP], b16, tag=f'hist{g}', name=f'hist{g}')
               for g in range(2)]
    half = (NKP // 2 // P) * P or NKP
    nc.sync.dma_start(out=sb_hist[0], in_=t_histTf.ap()[0:P, :])
    nc.scalar.dma_start(out=sb_hist[1], in_=t_histTf.ap()[P:2*P, :])
    sb_haug = sing.tile([NAUG, NKP], f32)
    hhalf = (NKP // 2 // 64) * 64 or NKP
    nc.gpsimd.dma_start(out=sb_haug[:, :hhalf], in_=t_haug.ap()[:, :hhalf])
    nc.sync.dma_start(out=sb_haug[:, hhalf:], in_=t_haug.ap()[:, hhalf:])
    sb_curT = [sing.tile([P, NQ], b16, tag=f'curT{g}', name=f'curT{g}')
               for g in range(2)]
    nc.scalar.dma_start(out=sb_curT[0], in_=t_curT.ap()[0:P, :])
    nc.gpsimd.dma_start(out=sb_curT[1], in_=t_curT.ap()[P:2*P, :])
    sb_qaug = sing.tile([NAUG, NQ], f32)
    nc.sync.dma_start(out=sb_qaug, in_=t_qaug.ap())
    sb_w = {}
    qs_ = [nc.sync, nc.scalar, nc.gpsimd]
    for qi, (nm, t) in enumerate((('q', t_wqT), ('k', t_wkT), ('v', t_wvT))):
        sb_w[nm] = [sing.tile([P, F], b16, tag=f'w{nm}{g}', name=f'w{nm}{g}')
                    for g in range(2)]
        for g in range(2):
            qs_[qi].dma_start(out=sb_w[nm][g], in_=t.ap()[g * P:(g + 1) * P, :])
    sb_woP = sing.tile([P, 4, F], b16)
    nc.sync.dma_start(out=sb_woP, in_=t_woP.ap())
    sb_bqk = sing.tile([P, 4], f32)
    nc.sync.dma_start(out=sb_bqk, in_=t_bqk.ap())
    sb_boT = sing.tile([P, 2], f32)
    nc.scalar.dma_start(out=sb_boT, in_=t_boT.ap())
    sb_bv = sing.tile([1, F], b16)
    nc.scalar.dma_start(out=sb_bv, in_=t_bv.ap())
    sb_one = sing.tile([1, P], b16)
    nc.vector.memset(sb_one, 1.0)
    sb_zero = sing.tile([1, 512], b16)
    nc.vector.memset(sb_zero, 0.0)

    # ---------------- PSUM layout
    d2t = ps_d2.tile([P, 512], f32, tag='d2', name='d2t')
    rb = ps_rb.tile([P, 512], f32, tag='rb', name='rb')
    nc.vector.memset(rb[D:64, :], 0.0)
    nc.vector.memset(rb[64 + D:128, :], 0.0)

    def proj_ps():
        return ps_sc.tile([P, 1024], f32, tag='sc', name='ps')

    # Q^T [f, q] (bf16), per f-half
    sb_QT = [sing.tile([P, NQ], b16, tag=f'QT{g}', name=f'QT{g}') for g in range(2)]
    for g in range(2):
        ps = proj_ps()
        for j in range(2):
            nc.tensor.matmul(ps[:, :NQ], sb_w['q'][j][:, g * P:(g + 1) * P],
                             sb_curT[j], start=(j == 0), stop=(j == 1))
        nc.scalar.activation(sb_QT[g], ps[:, :NQ], Ident, bias=sb_bqk[:, g:g + 1])
    # Masked Q for K=64 score matmuls: QM[g][c] has, in each 64-row block,
    # only the c-th 32-row half live (so head a uses block a//2 of QM[g][a%2]
    # and the other head sharing that block contributes zero).
    sb_QM = [[sing.tile([P, NQ], b16, tag=f'QM{g}{c}', name=f'QM{g}{c}')
              for c in range(2)] for g in range(2)]
    for g in range(2):
        for c in range(2):
            nc.vector.memset(sb_QM[g][c], 0.0)
            for b in range(2):
                r = 64 * b + 32 * c
                nc.vector.tensor_copy(sb_QM[g][c][r:r + 32, :],
                                      sb_QT[g][r:r + 32, :])
    # K^T [f, k] (bf16)
    sb_KT = [sing.tile([P, NKP], b16, tag=f'KT{g}', name=f'KT{g}') for g in range(2)]
    for g in range(2):
        for j4 in range(0, NCH, 4):
            w = min(4, NCH - j4) * P
            ps = proj_ps()
            for j in range(2):
                nc.tensor.matmul(ps[:, :w], sb_w['k'][j][:, g * P:(g + 1) * P],
                                 sb_hist[j][:, j4 * P:j4 * P + w],
                                 start=(j == 0), stop=(j == 1))
            nc.scalar.activation(sb_KT[g][:, j4 * P:j4 * P + w], ps[:, :w],
                                 Ident, bias=sb_bqk[:, 2 + g:3 + g])
    # V [k, h*33+d] (bf16) with ones column per head (preset once)
    sb_V = sing.tile([P, NCH, H * 33], b16)
    nc.vector.memset(sb_V.rearrange('p c (h x) -> p c h x', h=H)[:, :, :, D:D + 1],
                     1.0)
    for j in range(NCH):
        ps = proj_ps()
        for g in range(2):
            nc.tensor.matmul(ps[:, :F], sb_hist[g][:, j * P:(j + 1) * P],
                             sb_w['v'][g], start=(g == 0),
                             stop=(g == 1 and not vbias))
        if vbias:
            nc.tensor.matmul(ps[:, :F], sb_one[0:1, :P], sb_bv,
                             start=False, stop=True)
        vv = sb_V[:, j, :].rearrange('p (h x) -> p h x', h=H)
        pv = ps[:, :F].rearrange('p (h x) -> p h x', h=H)
        nc.vector.tensor_copy(vv[:, :, 0:D], pv)

    # ---------------- main loop: software-pipelined over all (slot, chunk)
    # PE stream per step t: [d2 group?] S(t); AV(t-2) — scores run 2 chunks
    # ahead of AV so the PE never stalls on the exp->mask chain.
    # normalized AV in (parity, block) layout: rows 64*(h%2)+d, block h//2
    sb_OP = sing.tile([P, 4, NQ], b16)
    nc.vector.memset(sb_OP, 0.0)
    av_tiles = {}

    chunks = [(s, j) for s in range(len(KW)) for j in range(KW[s])]
    n = len(chunks)
    m01s = {}     # (s, j // 4) -> mask tile
    sc_tiles = {}  # t -> per-chunk score psum tile
    e_tiles = {}   # t -> e tile
    pending_epi = []

    def emit_S(t):
        s, j = chunks[t]
        qsl = slice(s * QS, (s + 1) * QS)
        if j % 4 == 0:
            jn = min(4, KW[s] - j)
            for jj in range(jn):
                kc = (base[s] + j + jj) * P
                nc.tensor.matmul(d2t[:, jj * P:(jj + 1) * P],
                                 sb_haug[:, kc:kc + P], sb_qaug[:, qsl],
                                 start=True, stop=True)
            m01 = mpool.tile([P, 512], b16, tag='m', name='m01')
            nc.vector.tensor_scalar(out=m01[:, :jn * P], in0=d2t[:, :jn * P],
                                    scalar1=R2, scalar2=None,
                                    op0=mybir.AluOpType.is_le)
            m01s[(s, j // 4)] = m01
        kc = (base[s] + j) * P
        ksl = slice(kc, kc + P)
        # scores: 8 heads, K=64 half-masked, 2-way row-tiled: head (g, a)
        # contracts KT rows 64*(a//2)..+64 against QM[g][a%2]; row tiles at
        # {0, 64} write different banks of this chunk's 2-bank tile.
        sc = ps_sc.tile([P, 1024], f32, tag='sc', name='sc')
        scv = sc.rearrange('p (b g c q) -> p b g c q', b=2, g=2, c=2)
        for g in range(2):
            for a in (0, 2, 1, 3):
                b, c = a // 2, a % 2
                nc.tensor.matmul(
                    scv[:, b, g, c, :],
                    sb_KT[g][64 * b:64 * b + 64, ksl],
                    sb_QM[g][c][64 * b:64 * b + 64, qsl],
                    start=True, stop=True,
                    tile_position=(64 * b, 0))
        sc_tiles[t] = sc

    def emit_EM(t):
        s, j = chunks[t]
        sc = sc_tiles.pop(t)
        e = epool.tile([P, 2, 2, 2, P], b16, tag='e', name='e')
        nc.scalar.activation(e, sc, Exp, scale=ISCALE)
        ef = e.rearrange('p b g c q -> p (b g c) q')
        nc.vector.tensor_tensor(
            ef, ef,
            m01s[(s, j // 4)][:, None,
                              (j % 4) * P:(j % 4 + 1) * P].to_broadcast(
                [P, 8, P]),
            mybir.AluOpType.mult)
        e_tiles[t] = e

    def emit_AV(t):
        s, j = chunks[t]
        if j == 0:
            # zero the whole av bank and set every has_written bit so the 8
            # interleaved per-head accumulation chains can run start=False
            # (a per-head start=True would re-mark the whole 2KB zero region
            # and turn other heads' accumulations into overwrites).
            av = av_tiles[s] = ps_av.tile([P, 512], f32, tag='av', name='av')
            nc.tensor.matmul(av, sb_zero[0:1, 0:P], sb_zero[0:1, :],
                             start=True, stop=False, skip_group_check=True)
        av = av_tiles[s]
        e = e_tiles.pop(t)
        nkc = KW[s]
        for h in range(H):
            g, a = divmod(h, 4)
            po = 64 * (h % 2)
            fo = 128 * (h // 2)
            nc.tensor.matmul(
                av[po:po + 33, fo:fo + QS],
                sb_V[:, base[s] + j, 33 * h:33 * h + 33],
                e[:, a // 2, g, a % 2, :],
                start=False, stop=(j == nkc - 1 and h == H - 1),
                skip_group_check=True,
                tile_position=(0, po))
        if j == nkc - 1:
            pending_epi.append([s, 0])

    def emit_epilogue(s):
        qsl = slice(s * QS, (s + 1) * QS)
        av = av_tiles.pop(s)
        av_sb = opool.tile([P, 512], b16, tag='avsb', name='av_sb')
        nc.scalar.copy(av_sb, av)
        # reciprocal of the denominator rows (to partition 0 so the PE
        # broadcast matmul has lhsT/rhs at the same partition), broadcast
        # over the 32 head dims via PE in av (po, fo) layout, then multiply
        # with everything partition-aligned so the Pool engine can do it.
        rec = opool.tile([1, 1024], b16, tag='rec', name='rec')
        with nc.allow_low_precision(reason='softmax denom reciprocal in bf16; '
                                    'rel tol 2e-2 dominates'):
            nc.vector.reciprocal(rec[0:1, 0:512], av_sb[32:33, :])
            nc.vector.reciprocal(rec[0:1, 512:1024], av_sb[96:97, :])
        for h in range(H):
            po = 64 * (h % 2)
            fo = 128 * (h // 2)
            nc.tensor.matmul(rb[po:po + D, fo:fo + QS],
                             sb_one[0:1, 0:D],
                             rec[0:1, 512 * (h % 2) + fo:512 * (h % 2) + fo + QS],
                             start=True, stop=True,
                             tile_position=(0, po))
        rbs = mpool.tile([P, 512], b16, tag='rbs', name='rbs')
        nc.vector.tensor_copy(rbs, rb)
        for rho in range(2):
            nc.gpsimd.tensor_tensor(
                sb_OP[64 * rho:64 * rho + D, :, qsl],
                av_sb[64 * rho:64 * rho + D, :].rearrange(
                    'p (b q) -> p b q', b=4),
                rbs[64 * rho:64 * rho + D, :].rearrange(
                    'p (b q) -> p b q', b=4),
                mybir.AluOpType.mult)

    for t in range(n + 2):
        # age pending epilogues; emit once the slot's last AV is 2 steps old
        for ent in list(pending_epi):
            ent[1] += 1
            if ent[1] >= 2 or t >= n:
                emit_epilogue(ent[0])
                pending_epi.remove(ent)
        if t < n:
            emit_S(t)
        if 1 <= t <= n:
            emit_EM(t - 1)
        if t >= 2:
            emit_AV(t - 2)
    for ent in pending_epi:
        emit_epilogue(ent[0])

    # ---------------- output projection (K=128 per block; dead rows zero)
    for g2 in range(2):
        ps = proj_ps()
        for b in range(4):
            nc.tensor.matmul(ps[:, :NQ],
                             sb_woP[:, b, g2 * P:(g2 + 1) * P],
                             sb_OP[:, b, :],
                             start=(b == 0), stop=(b == 3))
        y = opool.tile([P, NQ], f32, tag='y', name='y')
        nc.scalar.activation(y, ps[:, :NQ], Ident, bias=sb_boT[:, g2:g2 + 1])
        (nc.sync if g2 == 0 else nc.scalar).dma_start(
            out=t_yT.ap()[g2 * P:(g2 + 1) * P, :], in_=y)


_CACHE = {}


def kernel(**inputs):
    from concourse import bass_utils

    in_maps, qmaps, KW, NKP, NAUG, vbias = _stage(inputs)
    key = (tuple(KW), NAUG, vbias)
    if key not in _CACHE:
        _CACHE[key] = _build(KW, NKP, NAUG, vbias)
    nc = _CACHE[key]
    res = bass_utils.run_bass_kernel_spmd(nc, in_maps, core_ids=list(range(NC)))
    N = inputs['current_feats'].shape[0]
    out = np.zeros((N, F), np.float32)
    for c in range(NC):
        out[qmaps[c]] = res.results[c]['yT'].T
    return out


if __name__ == '__main__':
    pass
